# revision 1
# baseline (speedup 1.0000x reference)
"""Depthwise 3x3 CNN combo kernel for TRN2 (8 NeuronCores, data-parallel).

Computes  out = relu(x*a0 + dwconv(x,w1)*a1 + dwconv(x,w2)*a2 + dwconv(x,w3)*a3)
for x [8, 256, 128, 128] f32, by folding everything into a single 9-tap
depthwise conv (conv is linear in the weights; the residual a0*x is the
center tap):  w_eff = a1*w1 + a2*w2 + a3*w3,  w_eff[:,1,1] += a0.

Sharding: batch dim across the 8 cores (one sample per core).

Per-core layout: channels on partitions (2 blocks of 128), image rows
padded to 130 columns (zero cols at 0 and 129) in the free dim, so every
tap (dy,dx) is a constant free-dim offset into the same SBUF tile.

Pipeline per chunk of 16 image rows:
  - HW-DGE DMA loads a compact f32 tile (contiguous ~9KB runs/partition).
  - ScalarE casts f32->bf16 into the padded tile; gpsimd zeroes pads.
  - Output produced in pairs of 4-row tiles (8 rows, N=1024):
      * TensorE pairs: 2x 9 accumulating diag-matmuls (bf16, 1 cyc/row)
        into f32 PSUM banks, relu'd by ScalarE into bf16 staging.
      * VectorE pairs: 9 fused scalar_tensor_tensor MACs on the whole
        8-row window into a bf16 SBUF accumulator, then relu.
  - Output DMA moves 8-row bf16 runs (2KB/partition); host upcasts.
"""

import numpy as np

import concourse.bacc as bacc
import concourse.mybir as mybir
from concourse import bass_utils
from concourse.tile import TileContext

# Problem constants (hardcoded per contract).
B = 8
C = 256
H = 128
W = 128
NCORES = 8

CB = 2           # channel blocks of 128
P = 128          # partitions
HC = 16          # image rows per chunk
NCHUNK = H // HC  # chunks per channel block
RT = 4           # output rows per PSUM tile (N = 4*128 = 512)
PAIR_ROWS = 2 * RT
PAIRS_PER_CHUNK = HC // PAIR_ROWS  # 2
WP = W + 2       # padded row width

TAPS = [(dy, dx) for dy in range(3) for dx in range(3)]

F32 = mybir.dt.float32
BF16 = mybir.dt.bfloat16


def _engine_for_pair(g):
    """Global pair index -> engine. 8-pair cycle: 6 PE, 2 DVE."""
    m = g % 8
    if m in (2, 5):
        return "dve"
    return "pe"


def _chunk_plan(first_cb):
    """Row chunks (h0, nrows). The first channel block starts with two
    8-row chunks so compute can begin after a small DMA+expand."""
    rows = [8, 8] + [16] * 7 if first_cb else [16] * 7 + [8, 8]
    plan, h0 = [], 0
    for r in rows:
        plan.append((h0, r))
        h0 += r
    return plan


def build_tile_kernel(tc, y_ap, x_ap, wdiag_ap, wvec_ap):
    nc = tc.nc
    relu = mybir.ActivationFunctionType.Relu
    copy = mybir.ActivationFunctionType.Copy
    mult, add = mybir.AluOpType.mult, mybir.AluOpType.add
    NW = RT * W          # 512
    PNW = PAIR_ROWS * W  # 1024

    with (
        tc.tile_pool(name="wpool", bufs=1) as wpool,
        tc.tile_pool(name="xcpool", bufs=4) as xcpool,
        tc.tile_pool(name="xpool", bufs=5) as xpool,
        tc.tile_pool(name="psum", bufs=8, space="PSUM") as psum_pool,
        tc.tile_pool(name="opool", bufs=6) as opool,
        tc.tile_pool(name="apool", bufs=3) as apool,
    ):
        # First chunk's input DMA goes out before the weight loads so the
        # critical path to the first expand starts immediately.
        xc0 = xcpool.tile([P, HC + 2, W], F32, tag="xc")
        first_hc = _chunk_plan(True)[0][1]
        nc.sync.dma_start(xc0[:, 1 : first_hc + 2, :], x_ap[0:P, 0 : first_hc + 1, :])

        # Per-block diagonal weight matrices for the PE: [k, cb, tap, m].
        # Weight loads go out on the ScalarE HW-DGE queue so they overlap
        # the first input chunk's DMA on the sync queue.
        wdiag = wpool.tile([P, CB, 9, P], BF16)
        nc.scalar.dma_start(wdiag[:], wdiag_ap)
        # Per-channel tap scalars for the DVE/GP: [c, cb, tap].
        wvec = wpool.tile([P, CB, 9], F32)
        nc.scalar.dma_start(wvec[:], wvec_ap)

        g = 0  # global pair counter
        for cb in range(CB):
            cs = slice(cb * P, (cb + 1) * P)
            for ci, (h0, hc) in enumerate(_chunk_plan(cb == 0)):
                r0 = 1 if h0 == 0 else 0
                r1 = (hc + 1) if h0 + hc == H else (hc + 2)
                # Padded bf16 tile: rows 0..hc+1 map to image rows
                # h0-1 .. h0+hc; cols 1..128 hold the image, cols 0/129 pad.
                xp = xpool.tile([P, HC + 2, WP], BF16, tag="xp")
                if cb == 0 and h0 == 0:
                    xc = xc0
                else:
                    # Compact f32 landing tile: contiguous HBM runs per
                    # partition keep the HW DGE at full bandwidth.
                    xc = xcpool.tile([P, HC + 2, W], F32, tag="xc")
                    nc.sync.dma_start(
                        xc[:, r0:r1, :],
                        x_ap[cs, h0 - 1 + r0 : h0 - 1 + r1, :],
                    )
                nc.scalar.activation(
                    xp[:, r0:r1, 1 : W + 1], xc[:, r0:r1, :], copy
                )
                nc.gpsimd.memset(xp[:, 0 : hc + 2, 0:1], 0.0)
                nc.gpsimd.memset(xp[:, 0 : hc + 2, W + 1 : W + 2], 0.0)
                if h0 == 0:
                    nc.gpsimd.memset(xp[:, 0:1, 1 : W + 1], 0.0)
                if h0 + hc == H:
                    nc.gpsimd.memset(xp[:, hc + 1 : hc + 2, 1 : W + 1], 0.0)

                for pj in range(hc // PAIR_ROWS):
                    jr = pj * PAIR_ROWS  # first output row (within chunk)
                    pair_sb = opool.tile([P, PNW], BF16)
                    engine = _engine_for_pair(g)
                    g += 1
                    if engine in ("dve", "gp"):
                        e = nc.vector if engine == "dve" else nc.gpsimd
                        # bf16 accumulator: enables the DVE 2x packed mode;
                        # rounding error verified < 1e-2 scaled on this data.
                        acc = apool.tile([P, PNW], BF16)
                        acc3 = acc[:].rearrange("p (r w) -> p r w", w=W)
                        for t, (dy, dx) in enumerate(TAPS):
                            rhs = xp[:, jr + dy : jr + dy + PAIR_ROWS, dx : dx + W]
                            sc = wvec[:, cb, t : t + 1]
                            if t == 0:
                                e.tensor_scalar_mul(acc3, rhs, sc)
                            else:
                                e.scalar_tensor_tensor(
                                    acc3, rhs, sc, acc3, mult, add
                                )
                        nc.scalar.activation(pair_sb[:], acc[:], relu)
                    else:
                        # Tap-major over the pair's two PSUM tiles: both
                        # halves of a tap run back-to-back on the PE with
                        # the same stationary weights, halving reloads.
                        psums = [
                            psum_pool.tile([P, NW], F32, name="ps", tag="ps")
                            for _ in range(2)
                        ]
                        for t, (dy, dx) in enumerate(TAPS):
                            for half in range(2):
                                hjr = jr + half * RT
                                rhs = xp[:, hjr + dy : hjr + dy + RT, dx : dx + W]
                                nc.tensor.matmul(
                                    psums[half][:],
                                    lhsT=wdiag[:, cb, t, :],
                                    rhs=rhs,
                                    start=(t == 0),
                                    stop=(t == 8),
                                    skip_group_check=True,
                                )
                        for half in range(2):
                            nc.scalar.activation(
                                pair_sb[:, half * NW : (half + 1) * NW],
                                psums[half][:],
                                relu,
                            )
                    nc.sync.dma_start(
                        y_ap[cs, h0 + jr : h0 + jr + PAIR_ROWS, :],
                        pair_sb[:].rearrange("p (r w) -> p r w", w=W),
                    )


def host_weights(a, w1, w2, w3):
    """Fold the 4-way combine into one 9-tap depthwise kernel; build the
    diag-matrix (PE) and per-channel-vector (DVE) forms."""
    a = np.asarray(a, np.float64)
    w_eff = (
        a[1] * np.asarray(w1, np.float64)[:, 0]
        + a[2] * np.asarray(w2, np.float64)[:, 0]
        + a[3] * np.asarray(w3, np.float64)[:, 0]
    )  # [C, 3, 3]
    w_eff[:, 1, 1] += a[0]
    wtap = w_eff.reshape(C, 9).astype(np.float32)

    import ml_dtypes

    wdiag = np.zeros((P, CB, 9, P), ml_dtypes.bfloat16)
    wvec = np.zeros((P, CB, 9), np.float32)
    idx = np.arange(P)
    for cb in range(CB):
        blk = wtap[cb * P : (cb + 1) * P]  # [128, 9]
        for t in range(9):
            wdiag[idx, cb, t, idx] = blk[:, t].astype(ml_dtypes.bfloat16)
        wvec[:, cb, :] = blk
    return wdiag, wvec


_PROGRAM = None


def _get_program():
    global _PROGRAM
    if _PROGRAM is None:
        nc = bacc.Bacc(
            "TRN2", target_bir_lowering=False, debug=False,
            enable_partition_id=False,
        )
        x_t = nc.dram_tensor("x", [C, H, W], F32, kind="ExternalInput")
        wdiag_t = nc.dram_tensor("wdiag", [P, CB, 9, P], BF16, kind="ExternalInput")
        wvec_t = nc.dram_tensor("wvec", [P, CB, 9], F32, kind="ExternalInput")
        y_t = nc.dram_tensor("y", [C, H, W], BF16, kind="ExternalOutput")
        with TileContext(nc) as tc:
            build_tile_kernel(tc, y_t.ap(), x_t.ap(), wdiag_t.ap(), wvec_t.ap())
        nc.compile()
        _PROGRAM = nc
    return _PROGRAM


def kernel(x, a, w1, w2, w3, _trace=False, _trace_kwargs=None):
    x = np.ascontiguousarray(np.asarray(x, np.float32))
    wdiag, wvec = host_weights(a, w1, w2, w3)
    in_maps = [
        {"x": x[i], "wdiag": wdiag, "wvec": wvec} for i in range(NCORES)
    ]
    nc = _get_program()
    res = bass_utils.run_bass_kernel_spmd(
        nc, in_maps, core_ids=list(range(NCORES)), trace=_trace,
        **(_trace_kwargs or {}),
    )
    out = np.stack([np.asarray(r["y"], np.float32) for r in res.results], axis=0)
    if _trace:
        return out, res
    return out

